# revision 20
# baseline (speedup 1.0000x reference)
"""LocalWindowAttention Trainium2 kernel.

Strategy: data-parallel over the 1024 (B*n_windows) windows -> 8 cores x 128
windows (2048 tokens each). Host pre-transposes x and the weights so every
matmul operand lands in SBUF with the contraction dim on partitions. All
matmuls run in bf16 (1 PE cycle/row); everything non-matmul is scheduled off
the PE critical path (ACT does PSUM->SBUF casts, DVE does RoPE/softmax glue,
GPSIMD issues output DMAs so they never block input prefetch on SP).

Per-core pipeline over 4 token blocks of 512:
  1. Q/K projections (PE) -> ACT casts PSUM to bf16 -> DVE RoPE (bf16 2x
     mode) -> QrT/KrT [hd, head, t].
  2. V projection (PE, bf16) interleaved with per-group attention softmax:
     scores matmul -> +mask in-PSUM (DVE) -> Exp with 1/sqrt(hd) folded into
     the activation scale (ACT) -> row-sum+recip+normalize (DVE) ->
     PE-transpose -> A^T staged to SBUF.
  3. AV matmuls + output projection interleaved per 128-token group so the
     PE never drains; weight chunks for block b+1 prefetch during block b
     from phase-aligned double-buffered pools.
"""

import json
import os
from functools import lru_cache

import numpy as np
import ml_dtypes

import concourse.bass as bass
import concourse.mybir as mybir
import concourse.tile as tile
from concourse.bass_utils import run_bass_kernel_spmd


def _split_waits_json(bir: bytes) -> bytes:
    """Walrus in this container embeds at most 1 sem-wait per instruction
    (2 for EventSemaphore). Tile freely attaches more. Spill the excess
    onto same-engine NoOps inserted right before the instruction."""
    j = json.loads(bir)
    ctr = [0]

    def cap_of(op):
        return 2 if op == "EventSemaphore" else 1

    for f in j["functions"]:
        for blk in f["blocks"]:
            out = []
            for inst in blk["instructions"]:
                si = inst.get("sync_info")
                waits = (si or {}).get("on_wait") or []
                cap = cap_of(inst.get("opcode"))
                if len(waits) > cap:
                    extra, keep = waits[:-cap], waits[-cap:]
                    for w in extra:
                        ctr[0] += 1
                        out.append({
                            "debug": inst.get("debug", 0),
                            "engine": inst["engine"],
                            "ins": [], "outs": [],
                            "name": f"I-wspill-{ctr[0]}",
                            "opcode": "NoOp",
                            "sync_info": {"on_update": [], "on_wait": [w]},
                        })
                    si["on_wait"] = keep
                out.append(inst)
            blk["instructions"] = out
    return json.dumps(j).encode()


def _patch_to_json(nc):
    orig = nc.to_json_bytes
    nc.to_json_bytes = lambda: _split_waits_json(orig())
    return nc

F32 = mybir.dt.float32
BF16 = mybir.dt.bfloat16
AX = mybir.AxisListType
ALU = mybir.AluOpType
ACTF = mybir.ActivationFunctionType

B, S, D = 4, 4096, 2048
H, HD, W = 16, 128, 16
E = H * HD  # 2048
NCORES = 8
TOK_PER_CORE = B * S // NCORES  # 2048
TBLK = 512            # tokens per block
NBLK = TOK_PER_CORE // TBLK  # 4
KT = D // 128         # 16 contraction tiles
ET = E // 128         # 16 e-tiles (= heads)
NG = TBLK // 128      # 4 groups (of 8 windows) per block
SCALE = 1.0 / float(np.sqrt(np.float32(HD)))
MASK_NEG = -30000.0 / SCALE  # so that exp(scale*(s+mask)) == 0 off-window


def build_kernel(nblk=NBLK):
    nc = bass.Bass("TRN2", target_bir_lowering=False, debug=False)

    ntok = nblk * TBLK
    # DRAM I/O (per core).
    xTb = nc.dram_tensor("xTb", [D, ntok], BF16, kind="ExternalInput")
    wqTb = nc.dram_tensor("wqTb", [D, E], BF16, kind="ExternalInput")
    wkTb = nc.dram_tensor("wkTb", [D, E], BF16, kind="ExternalInput")
    wvTb = nc.dram_tensor("wvTb", [D, E], BF16, kind="ExternalInput")
    woTb = nc.dram_tensor("woTb", [E, D], BF16, kind="ExternalInput")
    csd = nc.dram_tensor("csd", [128, TBLK], BF16, kind="ExternalInput")
    snd = nc.dram_tensor("snd", [128, TBLK], BF16, kind="ExternalInput")
    maskd = nc.dram_tensor("maskd", [128, 4, 128], BF16, kind="ExternalInput")
    idend = nc.dram_tensor("idend", [128, 128], BF16, kind="ExternalInput")
    outd = nc.dram_tensor("out", [ntok, D], BF16, kind="ExternalOutput")

    with tile.TileContext(nc) as tc:
        with (
            tc.tile_pool(name="const", bufs=1) as constp,
            tc.tile_pool(name="xb", bufs=2) as xbpool,
            tc.tile_pool(name="w", bufs=5) as wpool,
            tc.tile_pool(name="qk", bufs=1) as qkpool,
            tc.tile_pool(name="v", bufs=1) as vpool,
            tc.tile_pool(name="outT", bufs=1) as otpool,
            tc.tile_pool(name="rope", bufs=2) as ropep,
            tc.tile_pool(name="attn", bufs=2) as attnp,
            tc.tile_pool(name="ats", bufs=4) as atsp,
            tc.tile_pool(name="small", bufs=2) as smallp,
            tc.tile_pool(name="osb", bufs=2) as osbp,
            tc.tile_pool(name="psP", bufs=3, space="PSUM") as psP,
            tc.tile_pool(name="psS", bufs=3, space="PSUM") as psS,
            tc.tile_pool(name="psO", bufs=2, space="PSUM") as psO,
        ):
            def load_wchunk(pool, wdram, c):
                wt = pool.tile([128, KT, 512], BF16, tag="w")
                nc.sync.dma_start(
                    wt[:],
                    wdram[:, c * 512:(c + 1) * 512].rearrange(
                        "(k p) e -> p k e", p=128
                    ),
                )
                return wt

            def load_x(b):
                ts = b * TBLK
                xt = xbpool.tile([128, KT, TBLK], BF16, tag="xt")
                for kh in range(2):
                    ks = kh * (KT // 2)
                    nc.sync.dma_start(
                        xt[:, ks:ks + KT // 2, :],
                        xTb[ks * 128:(ks + KT // 2) * 128, ts:ts + TBLK]
                        .rearrange("(k p) t -> p k t", p=128),
                    )
                return xt

            # ---- startup: interleave x halves with a narrow first Q piece
            # (et0-1) so the PE can start ~4us in; consts sneak in before the
            # first RoPE needs them.
            ts0 = 0
            xt0 = xbpool.tile([128, KT, TBLK], BF16, tag="xt")
            nc.sync.dma_start(
                xt0[:, 0:KT // 2, :],
                xTb[0:(KT // 2) * 128, ts0:ts0 + TBLK]
                .rearrange("(k p) t -> p k t", p=128),
            )
            wq_first = wpool.tile([128, KT, 256], BF16, tag="w")
            nc.sync.dma_start(
                wq_first[:],
                wqTb[:, 0:256].rearrange("(k p) e -> p k e", p=128),
            )
            nc.sync.dma_start(
                xt0[:, KT // 2:KT, :],
                xTb[(KT // 2) * 128:D, ts0:ts0 + TBLK]
                .rearrange("(k p) t -> p k t", p=128),
            )
            wq0 = [load_wchunk(wpool, wqTb, 0)]
            cs_t = constp.tile([128, TBLK], BF16, tag="cs")
            sn_t = constp.tile([128, TBLK], BF16, tag="sn")
            nc.sync.dma_start(cs_t[:], csd[:])
            nc.sync.dma_start(sn_t[:], snd[:])
            for c in range(1, 4):
                wq0.append(load_wchunk(wpool, wqTb, c))
            mask = constp.tile([128, 4, 128], BF16, tag="mask")
            iden = constp.tile([128, 128], BF16, tag="iden")
            nc.sync.dma_start(mask[:], maskd[:])
            nc.sync.dma_start(iden[:], idend[:])
            wk0 = [load_wchunk(wpool, wkTb, c) for c in range(4)]
            wv0 = [load_wchunk(wpool, wvTb, c) for c in range(4)]
            wo0 = [load_wchunk(wpool, woTb, c) for c in range(4)]
            cur = dict(xt=xt0, wq=wq0, wk=wk0, wv=wv0, wo=wo0,
                       wq_first=wq_first)

            def emit_scores_softmax(g, qrt, krt, ats_g):
                """Softmax for group g -> A^T staged into ats_g [128,H,128]."""
                gs = g * 128
                for h0 in range(0, H, 4):
                    sps = psS.tile([128, 4, 128], F32, tag="s")
                    for i in range(4):
                        h = h0 + i
                        nc.tensor.matmul(
                            sps[:, i, :], qrt[:, h, gs:gs + 128],
                            krt[:, h, gs:gs + 128], start=True, stop=True)
                    sm = attnp.tile([128, 4, 128], BF16, tag="sm")
                    nc.vector.tensor_tensor(
                        out=sm[:], in0=sps[:], in1=mask[:], op=ALU.add)
                    pt = attnp.tile([128, 4, 128], BF16, tag="pt")
                    for i in range(4):
                        nc.scalar.activation(pt[:, i, :], sm[:, i, :],
                                             ACTF.Exp, scale=SCALE)
                    sums = smallp.tile([128, 4], F32, tag="sums")
                    nc.vector.reduce_sum(sums[:], pt[:], axis=AX.X)
                    rec = smallp.tile([128, 4], F32, tag="rec")
                    nc.vector.reciprocal(rec[:], sums[:])
                    for i in range(4):
                        nc.vector.tensor_scalar_mul(
                            pt[:, i, :], pt[:, i, :], rec[:, i:i + 1])
                    for i in range(4):
                        nc.scalar.dma_start_transpose(
                            ats_g[:, h0 + i, :], pt[:, i, :])

            for b in range(nblk):
                ts = b * TBLK
                xt = cur["xt"]

                # ---- Q/K projections + RoPE -> QrT/KrT bf16 [hd, head, t]
                qrt = qkpool.tile([128, ET, TBLK], BF16, tag="qrt")
                krt = qkpool.tile([128, ET, TBLK], BF16, tag="krt")
                for wname, dest in (("wq", qrt), ("wk", krt)):
                    wchunks = cur[wname]
                    for et in range(ET):
                        if wname == "wq" and "wq_first" in cur and et < 2:
                            wt = cur["wq_first"]
                            es = et * 128
                        else:
                            wt = wchunks[et // 4]
                            es = (et % 4) * 128
                        ps = psP.tile([128, TBLK], F32, tag="proj")
                        for k in range(KT):
                            nc.tensor.matmul(
                                ps[:], wt[:, k, es:es + 128], xt[:, k, :],
                                start=(k == 0), stop=(k == KT - 1),
                            )
                        # RoPE: dest = ps*cs + swap64(ps)*sn.  The
                        # partition-crossing reads MUST come from PSUM (HW
                        # forbids SB+SB operands with unequal base partition).
                        rot = ropep.tile([128, TBLK], BF16, tag="rot")
                        nc.vector.tensor_tensor(
                            out=rot[0:64, :], in0=ps[64:128, :],
                            in1=sn_t[0:64, :], op=ALU.mult)
                        nc.vector.tensor_tensor(
                            out=rot[64:128, :], in0=ps[0:64, :],
                            in1=sn_t[64:128, :], op=ALU.mult)
                        dv = dest[:, et, :]
                        nc.vector.tensor_tensor(
                            out=dv, in0=ps[:], in1=cs_t[:], op=ALU.mult)
                        nc.vector.tensor_tensor(
                            out=dv, in0=dv, in1=rot[:], op=ALU.add)

                # prefetch next block's inputs (wqk slots free during this
                # QK phase; wvo slots free during V/O phases)
                if b + 1 < nblk:
                    nxt = dict(
                        xt=load_x(b + 1),
                        wq=[load_wchunk(wpool, wqTb, c) for c in range(4)],
                        wk=[load_wchunk(wpool, wkTb, c) for c in range(4)],
                        wv=[load_wchunk(wpool, wvTb, c) for c in range(4)],
                        wo=[load_wchunk(wpool, woTb, c) for c in range(4)],
                    )
                else:
                    nxt = None

                # ---- V projection (PE) interleaved with attention softmax
                vt = vpool.tile([128, NG, E], BF16, tag="vt")
                ats_all = []
                for ec in range(4):
                    wv = cur["wv"][ec]
                    for tt in range(NG):
                        ps = psP.tile([128, TBLK], F32, tag="proj")
                        for k in range(KT):
                            nc.tensor.matmul(
                                ps[:], xt[:, k, tt * 128:(tt + 1) * 128],
                                wv[:, k, :],
                                start=(k == 0), stop=(k == KT - 1),
                            )
                        nc.scalar.copy(
                            vt[:, tt, ec * 512:(ec + 1) * 512], ps[:])
                    # softmax for group ec rides under the V matmuls
                    ats_g = atsp.tile([128, H, 128], BF16, tag="ats")
                    emit_scores_softmax(ec, qrt, krt, ats_g)
                    ats_all.append(ats_g)

                # ---- AV + output projection, interleaved per group
                outT = otpool.tile([128, ET, TBLK], BF16, tag="outT")

                def emit_av(g):
                    gs = g * 128
                    for h0 in range(0, H, 4):
                        ops_ = psO.tile([128, 4, 128], F32, tag="o")
                        for i in range(4):
                            h = h0 + i
                            nc.tensor.matmul(
                                ops_[:, i, :],
                                vt[:, g, h * 128:(h + 1) * 128],
                                ats_all[g][:, h, :], start=True, stop=True)
                        nc.scalar.copy(
                            outT[:, h0:h0 + 4, gs:gs + 128], ops_[:])

                def emit_oproj(tt):
                    for dc in range(4):
                        wo = cur["wo"][dc]
                        ps = psP.tile([128, TBLK], F32, tag="proj")
                        for et in range(ET):
                            nc.tensor.matmul(
                                ps[:], outT[:, et, tt * 128:(tt + 1) * 128],
                                wo[:, et, :],
                                start=(et == 0), stop=(et == ET - 1),
                            )
                        osb = osbp.tile([128, TBLK], BF16, tag="osb")
                        nc.scalar.copy(osb[:], ps[:])
                        nc.gpsimd.dma_start(
                            outd[ts + tt * 128: ts + (tt + 1) * 128,
                                 dc * 512:(dc + 1) * 512],
                            osb[:],
                        )

                emit_av(0)
                emit_av(1)
                emit_oproj(0)
                emit_av(2)
                emit_oproj(1)
                emit_av(3)
                emit_oproj(2)
                emit_oproj(3)

                cur = nxt
    return _patch_to_json(nc)


def _host_prep(x, rope_freqs, wq, wk, wv, wo):
    bf16 = ml_dtypes.bfloat16
    x_flat = np.ascontiguousarray(x, dtype=np.float32).reshape(B * S, D)
    wqTb = np.ascontiguousarray(wq.T).astype(bf16)
    wkTb = np.ascontiguousarray(wk.T).astype(bf16)
    wvTb = np.ascontiguousarray(wv.T).astype(bf16)
    woTb = np.ascontiguousarray(wo.T).astype(bf16)

    f = np.asarray(rope_freqs[:W], dtype=np.float32)  # [16, 64]
    cosf, sinf = np.cos(f), np.sin(f)                 # [16, 64]
    tmod = np.arange(TBLK) % W
    cs = np.empty((128, TBLK), np.float32)
    sn = np.empty((128, TBLK), np.float32)
    p = np.arange(128)
    cs[:, :] = cosf[tmod[None, :], (p % 64)[:, None]]
    sn[:, :] = sinf[tmod[None, :], (p % 64)[:, None]]
    sn[0:64, :] *= -1.0

    maskm = np.full((128, 128), MASK_NEG, np.float32)
    for wdw in range(128 // W):
        maskm[wdw * W:(wdw + 1) * W, wdw * W:(wdw + 1) * W] = 0.0
    maskm = np.repeat(maskm[:, None, :], 4, axis=1).copy()
    iden = np.eye(128, dtype=bf16)

    shared = dict(wqTb=wqTb, wkTb=wkTb, wvTb=wvTb, woTb=woTb,
                  csd=cs.astype(bf16), snd=sn.astype(bf16),
                  maskd=maskm.astype(bf16), idend=iden)
    in_maps = []
    for c in range(NCORES):
        shard = x_flat[c * TOK_PER_CORE:(c + 1) * TOK_PER_CORE]
        xT = np.ascontiguousarray(shard.T)
        in_maps.append(dict(shared, xTb=xT.astype(bf16)))
    return in_maps


@lru_cache(maxsize=1)
def _get_nc():
    return build_kernel()


def kernel(x, rope_freqs, wq, wk, wv, wo):
    in_maps = _host_prep(x, rope_freqs, wq, wk, wv, wo)
    nc = _get_nc()
    res = run_bass_kernel_spmd(
        nc, in_maps, core_ids=list(range(NCORES)),
        trace=bool(int(os.environ.get("LWA_TRACE", "0"))),
    )
    if getattr(kernel, "_last_results", None) is not None or True:
        kernel._last_results = res
    out = np.concatenate(
        [np.asarray(r["out"], dtype=np.float32) for r in res.results], axis=0)
    return out.reshape(B, S, D)


# revision 21
# speedup vs baseline: 1.1424x; 1.1424x over previous
"""LocalWindowAttention Trainium2 kernel.

Strategy: data-parallel over the 1024 (B*n_windows) windows -> 8 cores x 128
windows (2048 tokens each). Host pre-transposes x and the weights so every
matmul operand lands in SBUF with the contraction dim on partitions. All
matmuls run in bf16 (1 PE cycle/row); everything non-matmul is scheduled off
the PE critical path (ACT does PSUM->SBUF casts, DVE does RoPE/softmax glue,
GPSIMD issues output DMAs so they never block input prefetch on SP).

Per-core pipeline over 4 token blocks of 512:
  1. Q/K projections (PE) -> ACT casts PSUM to bf16 -> DVE RoPE (bf16 2x
     mode) -> QrT/KrT [hd, head, t].
  2. V projection (PE, bf16) interleaved with per-group attention softmax:
     scores matmul -> +mask in-PSUM (DVE) -> Exp with 1/sqrt(hd) folded into
     the activation scale (ACT) -> row-sum+recip+normalize (DVE) ->
     PE-transpose -> A^T staged to SBUF.
  3. AV matmuls + output projection interleaved per 128-token group so the
     PE never drains; weight chunks for block b+1 prefetch during block b
     from phase-aligned double-buffered pools.
"""

import json
import os
from functools import lru_cache

import numpy as np
import ml_dtypes

import concourse.bass as bass
import concourse.mybir as mybir
import concourse.tile as tile
from concourse.bass_utils import run_bass_kernel_spmd


def _split_waits_json(bir: bytes) -> bytes:
    """Walrus in this container embeds at most 1 sem-wait per instruction
    (2 for EventSemaphore). Tile freely attaches more. Spill the excess
    onto same-engine NoOps inserted right before the instruction."""
    j = json.loads(bir)
    ctr = [0]

    def cap_of(op):
        return 2 if op == "EventSemaphore" else 1

    for f in j["functions"]:
        for blk in f["blocks"]:
            out = []
            for inst in blk["instructions"]:
                si = inst.get("sync_info")
                waits = (si or {}).get("on_wait") or []
                cap = cap_of(inst.get("opcode"))
                if len(waits) > cap:
                    extra, keep = waits[:-cap], waits[-cap:]
                    for w in extra:
                        ctr[0] += 1
                        out.append({
                            "debug": inst.get("debug", 0),
                            "engine": inst["engine"],
                            "ins": [], "outs": [],
                            "name": f"I-wspill-{ctr[0]}",
                            "opcode": "NoOp",
                            "sync_info": {"on_update": [], "on_wait": [w]},
                        })
                    si["on_wait"] = keep
                out.append(inst)
            blk["instructions"] = out
    return json.dumps(j).encode()


def _patch_to_json(nc):
    orig = nc.to_json_bytes
    nc.to_json_bytes = lambda: _split_waits_json(orig())
    return nc

F32 = mybir.dt.float32
BF16 = mybir.dt.bfloat16
AX = mybir.AxisListType
ALU = mybir.AluOpType
ACTF = mybir.ActivationFunctionType

B, S, D = 4, 4096, 2048
H, HD, W = 16, 128, 16
E = H * HD  # 2048
NCORES = 8
TOK_PER_CORE = B * S // NCORES  # 2048
TBLK = 512            # tokens per block
NBLK = TOK_PER_CORE // TBLK  # 4
KT = D // 128         # 16 contraction tiles
ET = E // 128         # 16 e-tiles (= heads)
NG = TBLK // 128      # 4 groups (of 8 windows) per block
SCALE = 1.0 / float(np.sqrt(np.float32(HD)))
MASK_NEG = -30000.0 / SCALE  # so that exp(scale*(s+mask)) == 0 off-window


def build_kernel(nblk=NBLK):
    nc = bass.Bass("TRN2", target_bir_lowering=False, debug=False)

    ntok = nblk * TBLK
    # DRAM I/O (per core).
    xTb = nc.dram_tensor("xTb", [D, ntok], BF16, kind="ExternalInput")
    wqTb = nc.dram_tensor("wqTb", [D, E], BF16, kind="ExternalInput")
    wkTb = nc.dram_tensor("wkTb", [D, E], BF16, kind="ExternalInput")
    wvTb = nc.dram_tensor("wvTb", [D, E], BF16, kind="ExternalInput")
    woTb = nc.dram_tensor("woTb", [E, D], BF16, kind="ExternalInput")
    csd = nc.dram_tensor("csd", [128, TBLK], BF16, kind="ExternalInput")
    snd = nc.dram_tensor("snd", [128, TBLK], BF16, kind="ExternalInput")
    maskd = nc.dram_tensor("maskd", [128, 4, 128], BF16, kind="ExternalInput")
    idend = nc.dram_tensor("idend", [128, 128], BF16, kind="ExternalInput")
    outd = nc.dram_tensor("out", [ntok, D], BF16, kind="ExternalOutput")

    with tile.TileContext(nc) as tc:
        with (
            tc.tile_pool(name="const", bufs=1) as constp,
            tc.tile_pool(name="xb", bufs=2) as xbpool,
            tc.tile_pool(name="w", bufs=5) as wpool,
            tc.tile_pool(name="qk", bufs=1) as qkpool,
            tc.tile_pool(name="v", bufs=1) as vpool,
            tc.tile_pool(name="outT", bufs=1) as otpool,
            tc.tile_pool(name="rope", bufs=2) as ropep,
            tc.tile_pool(name="attn", bufs=2) as attnp,
            tc.tile_pool(name="ats", bufs=4) as atsp,
            tc.tile_pool(name="small", bufs=2) as smallp,
            tc.tile_pool(name="osb", bufs=2) as osbp,
            tc.tile_pool(name="psP", bufs=2, space="PSUM") as psP,
            tc.tile_pool(name="psS", bufs=3, space="PSUM") as psS,
            tc.tile_pool(name="psT", bufs=1, space="PSUM") as psT,
            tc.tile_pool(name="psO", bufs=2, space="PSUM") as psO,
        ):
            def load_wchunk(pool, wdram, c):
                wt = pool.tile([128, KT, 512], BF16, tag="w")
                nc.sync.dma_start(
                    wt[:],
                    wdram[:, c * 512:(c + 1) * 512].rearrange(
                        "(k p) e -> p k e", p=128
                    ),
                )
                return wt

            def load_x(b):
                ts = b * TBLK
                xt = xbpool.tile([128, KT, TBLK], BF16, tag="xt")
                for kh in range(2):
                    ks = kh * (KT // 2)
                    nc.sync.dma_start(
                        xt[:, ks:ks + KT // 2, :],
                        xTb[ks * 128:(ks + KT // 2) * 128, ts:ts + TBLK]
                        .rearrange("(k p) t -> p k t", p=128),
                    )
                return xt

            # ---- startup: interleave x halves with a narrow first Q piece
            # (et0-1) so the PE can start ~4us in; consts sneak in before the
            # first RoPE needs them.
            ts0 = 0
            xt0 = xbpool.tile([128, KT, TBLK], BF16, tag="xt")
            nc.sync.dma_start(
                xt0[:, 0:KT // 2, :],
                xTb[0:(KT // 2) * 128, ts0:ts0 + TBLK]
                .rearrange("(k p) t -> p k t", p=128),
            )
            wq_first = wpool.tile([128, KT, 256], BF16, tag="w")
            nc.sync.dma_start(
                wq_first[:],
                wqTb[:, 0:256].rearrange("(k p) e -> p k e", p=128),
            )
            nc.sync.dma_start(
                xt0[:, KT // 2:KT, :],
                xTb[(KT // 2) * 128:D, ts0:ts0 + TBLK]
                .rearrange("(k p) t -> p k t", p=128),
            )
            wq0 = [load_wchunk(wpool, wqTb, 0)]
            cs_t = constp.tile([128, TBLK], BF16, tag="cs")
            sn_t = constp.tile([128, TBLK], BF16, tag="sn")
            nc.sync.dma_start(cs_t[:], csd[:])
            nc.sync.dma_start(sn_t[:], snd[:])
            for c in range(1, 4):
                wq0.append(load_wchunk(wpool, wqTb, c))
            mask = constp.tile([128, 4, 128], BF16, tag="mask")
            iden = constp.tile([128, 128], BF16, tag="iden")
            nc.sync.dma_start(mask[:], maskd[:])
            nc.sync.dma_start(iden[:], idend[:])
            wk0 = [load_wchunk(wpool, wkTb, c) for c in range(4)]
            wv0 = [load_wchunk(wpool, wvTb, c) for c in range(4)]
            wo0 = [load_wchunk(wpool, woTb, c) for c in range(4)]
            cur = dict(xt=xt0, wq=wq0, wk=wk0, wv=wv0, wo=wo0,
                       wq_first=wq_first)

            def emit_scores_softmax(g, qrt, krt, ats_g):
                """Softmax for group g -> A^T staged into ats_g [128,H,128]."""
                gs = g * 128
                for h0 in range(0, H, 4):
                    sps = psS.tile([128, 4, 128], F32, tag="s")
                    for i in range(4):
                        h = h0 + i
                        nc.tensor.matmul(
                            sps[:, i, :], qrt[:, h, gs:gs + 128],
                            krt[:, h, gs:gs + 128], start=True, stop=True)
                    sm = attnp.tile([128, 4, 128], BF16, tag="sm")
                    nc.vector.tensor_tensor(
                        out=sm[:], in0=sps[:], in1=mask[:], op=ALU.add)
                    pt = attnp.tile([128, 4, 128], BF16, tag="pt")
                    for i in range(4):
                        nc.scalar.activation(pt[:, i, :], sm[:, i, :],
                                             ACTF.Exp, scale=SCALE)
                    sums = smallp.tile([128, 4], F32, tag="sums")
                    nc.vector.reduce_sum(sums[:], pt[:], axis=AX.X)
                    rec = smallp.tile([128, 4], F32, tag="rec")
                    nc.vector.reciprocal(rec[:], sums[:])
                    for i in range(4):
                        nc.vector.tensor_scalar_mul(
                            pt[:, i, :], pt[:, i, :], rec[:, i:i + 1])
                    atps = psT.tile([128, 4, 128], BF16, tag="t")
                    for i in range(4):
                        nc.tensor.transpose(atps[:, i, :], pt[:, i, :],
                                            iden[:])
                    nc.vector.tensor_copy(ats_g[:, h0:h0 + 4, :], atps[:])

            for b in range(nblk):
                ts = b * TBLK
                xt = cur["xt"]

                # ---- Q/K projections + RoPE -> QrT/KrT bf16 [hd, head, t]
                qrt = qkpool.tile([128, ET, TBLK], BF16, tag="qrt")
                krt = qkpool.tile([128, ET, TBLK], BF16, tag="krt")
                for wname, dest in (("wq", qrt), ("wk", krt)):
                    wchunks = cur[wname]
                    for et in range(ET):
                        if wname == "wq" and "wq_first" in cur and et < 2:
                            wt = cur["wq_first"]
                            es = et * 128
                        else:
                            wt = wchunks[et // 4]
                            es = (et % 4) * 128
                        ps = psP.tile([128, TBLK], F32, tag="proj")
                        for k in range(KT):
                            nc.tensor.matmul(
                                ps[:], wt[:, k, es:es + 128], xt[:, k, :],
                                start=(k == 0), stop=(k == KT - 1),
                            )
                        # RoPE: dest = ps*cs + swap64(ps)*sn.  The
                        # partition-crossing reads MUST come from PSUM (HW
                        # forbids SB+SB operands with unequal base partition).
                        rot = ropep.tile([128, TBLK], BF16, tag="rot")
                        nc.vector.tensor_tensor(
                            out=rot[0:64, :], in0=ps[64:128, :],
                            in1=sn_t[0:64, :], op=ALU.mult)
                        nc.vector.tensor_tensor(
                            out=rot[64:128, :], in0=ps[0:64, :],
                            in1=sn_t[64:128, :], op=ALU.mult)
                        dv = dest[:, et, :]
                        nc.vector.tensor_tensor(
                            out=dv, in0=ps[:], in1=cs_t[:], op=ALU.mult)
                        nc.vector.tensor_tensor(
                            out=dv, in0=dv, in1=rot[:], op=ALU.add)

                # prefetch next block's inputs (wqk slots free during this
                # QK phase; wvo slots free during V/O phases)
                if b + 1 < nblk:
                    nxt = dict(
                        xt=load_x(b + 1),
                        wq=[load_wchunk(wpool, wqTb, c) for c in range(4)],
                        wk=[load_wchunk(wpool, wkTb, c) for c in range(4)],
                        wv=[load_wchunk(wpool, wvTb, c) for c in range(4)],
                        wo=[load_wchunk(wpool, woTb, c) for c in range(4)],
                    )
                else:
                    nxt = None

                # ---- V projection (PE) interleaved with attention softmax
                vt = vpool.tile([128, NG, E], BF16, tag="vt")
                ats_all = []
                for ec in range(4):
                    wv = cur["wv"][ec]
                    for tt in range(NG):
                        ps = psP.tile([128, TBLK], F32, tag="proj")
                        for k in range(KT):
                            nc.tensor.matmul(
                                ps[:], xt[:, k, tt * 128:(tt + 1) * 128],
                                wv[:, k, :],
                                start=(k == 0), stop=(k == KT - 1),
                            )
                        nc.scalar.copy(
                            vt[:, tt, ec * 512:(ec + 1) * 512], ps[:])
                    # softmax for group ec rides under the V matmuls
                    ats_g = atsp.tile([128, H, 128], BF16, tag="ats")
                    emit_scores_softmax(ec, qrt, krt, ats_g)
                    ats_all.append(ats_g)

                # ---- AV + output projection, interleaved per group
                outT = otpool.tile([128, ET, TBLK], BF16, tag="outT")

                def emit_av(g):
                    gs = g * 128
                    for h0 in range(0, H, 4):
                        ops_ = psO.tile([128, 4, 128], F32, tag="o")
                        for i in range(4):
                            h = h0 + i
                            nc.tensor.matmul(
                                ops_[:, i, :],
                                vt[:, g, h * 128:(h + 1) * 128],
                                ats_all[g][:, h, :], start=True, stop=True)
                        nc.scalar.copy(
                            outT[:, h0:h0 + 4, gs:gs + 128], ops_[:])

                def emit_oproj(tt):
                    for dc in range(4):
                        wo = cur["wo"][dc]
                        ps = psP.tile([128, TBLK], F32, tag="proj")
                        for et in range(ET):
                            nc.tensor.matmul(
                                ps[:], outT[:, et, tt * 128:(tt + 1) * 128],
                                wo[:, et, :],
                                start=(et == 0), stop=(et == ET - 1),
                            )
                        osb = osbp.tile([128, TBLK], BF16, tag="osb")
                        nc.scalar.copy(osb[:], ps[:])
                        nc.gpsimd.dma_start(
                            outd[ts + tt * 128: ts + (tt + 1) * 128,
                                 dc * 512:(dc + 1) * 512],
                            osb[:],
                        )

                emit_av(0)
                emit_av(1)
                emit_oproj(0)
                emit_av(2)
                emit_oproj(1)
                emit_av(3)
                emit_oproj(2)
                emit_oproj(3)

                cur = nxt
    return _patch_to_json(nc)


def _host_prep(x, rope_freqs, wq, wk, wv, wo):
    bf16 = ml_dtypes.bfloat16
    x_flat = np.ascontiguousarray(x, dtype=np.float32).reshape(B * S, D)
    wqTb = np.ascontiguousarray(wq.T).astype(bf16)
    wkTb = np.ascontiguousarray(wk.T).astype(bf16)
    wvTb = np.ascontiguousarray(wv.T).astype(bf16)
    woTb = np.ascontiguousarray(wo.T).astype(bf16)

    f = np.asarray(rope_freqs[:W], dtype=np.float32)  # [16, 64]
    cosf, sinf = np.cos(f), np.sin(f)                 # [16, 64]
    tmod = np.arange(TBLK) % W
    cs = np.empty((128, TBLK), np.float32)
    sn = np.empty((128, TBLK), np.float32)
    p = np.arange(128)
    cs[:, :] = cosf[tmod[None, :], (p % 64)[:, None]]
    sn[:, :] = sinf[tmod[None, :], (p % 64)[:, None]]
    sn[0:64, :] *= -1.0

    maskm = np.full((128, 128), MASK_NEG, np.float32)
    for wdw in range(128 // W):
        maskm[wdw * W:(wdw + 1) * W, wdw * W:(wdw + 1) * W] = 0.0
    maskm = np.repeat(maskm[:, None, :], 4, axis=1).copy()
    iden = np.eye(128, dtype=bf16)

    shared = dict(wqTb=wqTb, wkTb=wkTb, wvTb=wvTb, woTb=woTb,
                  csd=cs.astype(bf16), snd=sn.astype(bf16),
                  maskd=maskm.astype(bf16), idend=iden)
    in_maps = []
    for c in range(NCORES):
        shard = x_flat[c * TOK_PER_CORE:(c + 1) * TOK_PER_CORE]
        xT = np.ascontiguousarray(shard.T)
        in_maps.append(dict(shared, xTb=xT.astype(bf16)))
    return in_maps


@lru_cache(maxsize=1)
def _get_nc():
    return build_kernel()


def kernel(x, rope_freqs, wq, wk, wv, wo):
    in_maps = _host_prep(x, rope_freqs, wq, wk, wv, wo)
    nc = _get_nc()
    res = run_bass_kernel_spmd(
        nc, in_maps, core_ids=list(range(NCORES)),
        trace=bool(int(os.environ.get("LWA_TRACE", "0"))),
    )
    if getattr(kernel, "_last_results", None) is not None or True:
        kernel._last_results = res
    out = np.concatenate(
        [np.asarray(r["out"], dtype=np.float32) for r in res.results], axis=0)
    return out.reshape(B, S, D)


# revision 22
# speedup vs baseline: 1.1572x; 1.0130x over previous
"""LocalWindowAttention Trainium2 kernel.

Strategy: data-parallel over the 1024 (B*n_windows) windows -> 8 cores x 128
windows (2048 tokens each). Host pre-transposes x and the weights so every
matmul operand lands in SBUF with the contraction dim on partitions. All
matmuls run in bf16 (1 PE cycle/row); everything non-matmul is scheduled off
the PE critical path (ACT does PSUM->SBUF casts, DVE does RoPE/softmax glue,
GPSIMD issues output DMAs so they never block input prefetch on SP).

Per-core pipeline over 4 token blocks of 512:
  1. Q/K projections (PE) -> ACT casts PSUM to bf16 -> DVE RoPE (bf16 2x
     mode) -> QrT/KrT [hd, head, t].
  2. V projection (PE, bf16) interleaved with per-group attention softmax:
     scores matmul -> +mask in-PSUM (DVE) -> Exp with 1/sqrt(hd) folded into
     the activation scale (ACT) -> row-sum+recip+normalize (DVE) ->
     PE-transpose -> A^T staged to SBUF.
  3. AV matmuls + output projection interleaved per 128-token group so the
     PE never drains; weight chunks for block b+1 prefetch during block b
     from phase-aligned double-buffered pools.
"""

import json
import os
from functools import lru_cache

import numpy as np
import ml_dtypes

import concourse.bass as bass
import concourse.mybir as mybir
import concourse.tile as tile
from concourse.bass_utils import run_bass_kernel_spmd


def _split_waits_json(bir: bytes) -> bytes:
    """Walrus in this container embeds at most 1 sem-wait per instruction
    (2 for EventSemaphore). Tile freely attaches more. Spill the excess
    onto same-engine NoOps inserted right before the instruction."""
    j = json.loads(bir)
    ctr = [0]

    def cap_of(op):
        return 2 if op == "EventSemaphore" else 1

    for f in j["functions"]:
        for blk in f["blocks"]:
            out = []
            for inst in blk["instructions"]:
                si = inst.get("sync_info")
                waits = (si or {}).get("on_wait") or []
                cap = cap_of(inst.get("opcode"))
                if len(waits) > cap:
                    extra, keep = waits[:-cap], waits[-cap:]
                    for w in extra:
                        ctr[0] += 1
                        out.append({
                            "debug": inst.get("debug", 0),
                            "engine": inst["engine"],
                            "ins": [], "outs": [],
                            "name": f"I-wspill-{ctr[0]}",
                            "opcode": "NoOp",
                            "sync_info": {"on_update": [], "on_wait": [w]},
                        })
                    si["on_wait"] = keep
                out.append(inst)
            blk["instructions"] = out
    return json.dumps(j).encode()


def _patch_to_json(nc):
    orig = nc.to_json_bytes
    nc.to_json_bytes = lambda: _split_waits_json(orig())
    return nc

F32 = mybir.dt.float32
BF16 = mybir.dt.bfloat16
AX = mybir.AxisListType
ALU = mybir.AluOpType
ACTF = mybir.ActivationFunctionType

B, S, D = 4, 4096, 2048
H, HD, W = 16, 128, 16
E = H * HD  # 2048
NCORES = 8
TOK_PER_CORE = B * S // NCORES  # 2048
TBLK = 512            # tokens per block
NBLK = TOK_PER_CORE // TBLK  # 4
KT = D // 128         # 16 contraction tiles
ET = E // 128         # 16 e-tiles (= heads)
NG = TBLK // 128      # 4 groups (of 8 windows) per block
SCALE = 1.0 / float(np.sqrt(np.float32(HD)))
MASK_NEG = -30000.0 / SCALE  # so that exp(scale*(s+mask)) == 0 off-window


def build_kernel(nblk=NBLK):
    nc = bass.Bass("TRN2", target_bir_lowering=False, debug=False)

    ntok = nblk * TBLK
    # DRAM I/O (per core).
    xTb = nc.dram_tensor("xTb", [D, ntok], BF16, kind="ExternalInput")
    wqTb = nc.dram_tensor("wqTb", [D, E], BF16, kind="ExternalInput")
    wkTb = nc.dram_tensor("wkTb", [D, E], BF16, kind="ExternalInput")
    wvTb = nc.dram_tensor("wvTb", [D, E], BF16, kind="ExternalInput")
    woTb = nc.dram_tensor("woTb", [E, D], BF16, kind="ExternalInput")
    csd = nc.dram_tensor("csd", [128, TBLK], BF16, kind="ExternalInput")
    snd = nc.dram_tensor("snd", [128, TBLK], BF16, kind="ExternalInput")
    maskd = nc.dram_tensor("maskd", [128, 4, 128], BF16, kind="ExternalInput")
    idend = nc.dram_tensor("idend", [128, 128], BF16, kind="ExternalInput")
    outd = nc.dram_tensor("out", [ntok, D], BF16, kind="ExternalOutput")

    with tile.TileContext(nc) as tc:
        with (
            tc.tile_pool(name="const", bufs=1) as constp,
            tc.tile_pool(name="xb", bufs=2) as xbpool,
            tc.tile_pool(name="w", bufs=5) as wpool,
            tc.tile_pool(name="qk", bufs=1) as qkpool,
            tc.tile_pool(name="v", bufs=1) as vpool,
            tc.tile_pool(name="outT", bufs=1) as otpool,
            tc.tile_pool(name="rope", bufs=2) as ropep,
            tc.tile_pool(name="attn", bufs=2) as attnp,
            tc.tile_pool(name="ats", bufs=4) as atsp,
            tc.tile_pool(name="small", bufs=2) as smallp,
            tc.tile_pool(name="osb", bufs=2) as osbp,
            tc.tile_pool(name="psP", bufs=3, space="PSUM") as psP,
            tc.tile_pool(name="psS", bufs=2, space="PSUM") as psS,
            tc.tile_pool(name="psT", bufs=1, space="PSUM") as psT,
            tc.tile_pool(name="psO", bufs=2, space="PSUM") as psO,
        ):
            def load_wchunk(pool, wdram, c):
                wt = pool.tile([128, KT, 512], BF16, tag="w")
                nc.sync.dma_start(
                    wt[:],
                    wdram[:, c * 512:(c + 1) * 512].rearrange(
                        "(k p) e -> p k e", p=128
                    ),
                )
                return wt

            def load_x(b):
                ts = b * TBLK
                xt = xbpool.tile([128, KT, TBLK], BF16, tag="xt")
                for kh in range(2):
                    ks = kh * (KT // 2)
                    nc.sync.dma_start(
                        xt[:, ks:ks + KT // 2, :],
                        xTb[ks * 128:(ks + KT // 2) * 128, ts:ts + TBLK]
                        .rearrange("(k p) t -> p k t", p=128),
                    )
                return xt

            # ---- startup: interleave x halves with a narrow first Q piece
            # (et0-1) so the PE can start ~4us in; consts sneak in before the
            # first RoPE needs them.
            ts0 = 0
            xt0 = xbpool.tile([128, KT, TBLK], BF16, tag="xt")
            nc.sync.dma_start(
                xt0[:, 0:KT // 2, :],
                xTb[0:(KT // 2) * 128, ts0:ts0 + TBLK]
                .rearrange("(k p) t -> p k t", p=128),
            )
            wq_first = wpool.tile([128, KT, 256], BF16, tag="w")
            nc.sync.dma_start(
                wq_first[:],
                wqTb[:, 0:256].rearrange("(k p) e -> p k e", p=128),
            )
            nc.sync.dma_start(
                xt0[:, KT // 2:KT, :],
                xTb[(KT // 2) * 128:D, ts0:ts0 + TBLK]
                .rearrange("(k p) t -> p k t", p=128),
            )
            wq0 = [load_wchunk(wpool, wqTb, 0)]
            cs_t = constp.tile([128, TBLK], BF16, tag="cs")
            sn_t = constp.tile([128, TBLK], BF16, tag="sn")
            nc.sync.dma_start(cs_t[:], csd[:])
            nc.sync.dma_start(sn_t[:], snd[:])
            for c in range(1, 4):
                wq0.append(load_wchunk(wpool, wqTb, c))
            mask = constp.tile([128, 4, 128], BF16, tag="mask")
            iden = constp.tile([128, 128], BF16, tag="iden")
            nc.sync.dma_start(mask[:], maskd[:])
            nc.sync.dma_start(iden[:], idend[:])
            wk0 = [load_wchunk(wpool, wkTb, c) for c in range(4)]
            wv0 = [load_wchunk(wpool, wvTb, c) for c in range(4)]
            wo0 = [load_wchunk(wpool, woTb, c) for c in range(4)]
            cur = dict(xt=xt0, wq=wq0, wk=wk0, wv=wv0, wo=wo0,
                       wq_first=wq_first)

            def emit_scores_softmax(g, qrt, krt, ats_g):
                """Softmax for group g -> A^T staged into ats_g [128,H,128]."""
                gs = g * 128
                for h0 in range(0, H, 4):
                    sps = psS.tile([128, 4, 128], F32, tag="s")
                    for i in range(4):
                        h = h0 + i
                        nc.tensor.matmul(
                            sps[:, i, :], qrt[:, h, gs:gs + 128],
                            krt[:, h, gs:gs + 128], start=True, stop=True)
                    sm = attnp.tile([128, 4, 128], BF16, tag="sm")
                    nc.vector.tensor_tensor(
                        out=sm[:], in0=sps[:], in1=mask[:], op=ALU.add)
                    pt = attnp.tile([128, 4, 128], BF16, tag="pt")
                    for i in range(4):
                        nc.scalar.activation(pt[:, i, :], sm[:, i, :],
                                             ACTF.Exp, scale=SCALE)
                    sums = smallp.tile([128, 4], F32, tag="sums")
                    nc.vector.reduce_sum(sums[:], pt[:], axis=AX.X)
                    rec = smallp.tile([128, 4], F32, tag="rec")
                    nc.vector.reciprocal(rec[:], sums[:])
                    for i in range(4):
                        nc.vector.tensor_scalar_mul(
                            pt[:, i, :], pt[:, i, :], rec[:, i:i + 1])
                    atps = psT.tile([128, 4, 128], BF16, tag="t")
                    for i in range(4):
                        nc.tensor.transpose(atps[:, i, :], pt[:, i, :],
                                            iden[:])
                    nc.vector.tensor_copy(ats_g[:, h0:h0 + 4, :], atps[:])

            for b in range(nblk):
                ts = b * TBLK
                xt = cur["xt"]

                # ---- Q/K projections + RoPE -> QrT/KrT bf16 [hd, head, t]
                qrt = qkpool.tile([128, ET, TBLK], BF16, tag="qrt")
                krt = qkpool.tile([128, ET, TBLK], BF16, tag="krt")
                for wname, dest in (("wq", qrt), ("wk", krt)):
                    wchunks = cur[wname]
                    for et in range(ET):
                        if wname == "wq" and "wq_first" in cur and et < 2:
                            wt = cur["wq_first"]
                            es = et * 128
                        else:
                            wt = wchunks[et // 4]
                            es = (et % 4) * 128
                        ps = psP.tile([128, TBLK], F32, tag="proj")
                        for k in range(KT):
                            nc.tensor.matmul(
                                ps[:], wt[:, k, es:es + 128], xt[:, k, :],
                                start=(k == 0), stop=(k == KT - 1),
                            )
                        # RoPE: dest = ps*cs + swap64(ps)*sn.  The
                        # partition-crossing reads MUST come from PSUM (HW
                        # forbids SB+SB operands with unequal base partition).
                        rot = ropep.tile([128, TBLK], BF16, tag="rot")
                        nc.vector.tensor_tensor(
                            out=rot[0:64, :], in0=ps[64:128, :],
                            in1=sn_t[0:64, :], op=ALU.mult)
                        nc.vector.tensor_tensor(
                            out=rot[64:128, :], in0=ps[0:64, :],
                            in1=sn_t[64:128, :], op=ALU.mult)
                        dv = dest[:, et, :]
                        nc.vector.tensor_tensor(
                            out=dv, in0=ps[:], in1=cs_t[:], op=ALU.mult)
                        nc.vector.tensor_tensor(
                            out=dv, in0=dv, in1=rot[:], op=ALU.add)

                # prefetch next block's inputs (wqk slots free during this
                # QK phase; wvo slots free during V/O phases)
                if b + 1 < nblk:
                    nxt = dict(
                        xt=load_x(b + 1),
                        wq=[load_wchunk(wpool, wqTb, c) for c in range(4)],
                        wk=[load_wchunk(wpool, wkTb, c) for c in range(4)],
                        wv=[load_wchunk(wpool, wvTb, c) for c in range(4)],
                        wo=[load_wchunk(wpool, woTb, c) for c in range(4)],
                    )
                else:
                    nxt = None

                # ---- V projection (PE) interleaved with attention softmax
                vt = vpool.tile([128, NG, E], BF16, tag="vt")
                ats_all = []
                for ec in range(4):
                    wv = cur["wv"][ec]
                    for tt in range(NG):
                        ps = psP.tile([128, TBLK], F32, tag="proj")
                        for k in range(KT):
                            nc.tensor.matmul(
                                ps[:], xt[:, k, tt * 128:(tt + 1) * 128],
                                wv[:, k, :],
                                start=(k == 0), stop=(k == KT - 1),
                            )
                        nc.scalar.copy(
                            vt[:, tt, ec * 512:(ec + 1) * 512], ps[:])
                    # softmax for group ec rides under the V matmuls
                    ats_g = atsp.tile([128, H, 128], BF16, tag="ats")
                    emit_scores_softmax(ec, qrt, krt, ats_g)
                    ats_all.append(ats_g)

                # ---- AV + output projection, interleaved per group
                outT = otpool.tile([128, ET, TBLK], BF16, tag="outT")

                def emit_av(g):
                    gs = g * 128
                    for h0 in range(0, H, 4):
                        ops_ = psO.tile([128, 4, 128], F32, tag="o")
                        for i in range(4):
                            h = h0 + i
                            nc.tensor.matmul(
                                ops_[:, i, :],
                                vt[:, g, h * 128:(h + 1) * 128],
                                ats_all[g][:, h, :], start=True, stop=True)
                        nc.scalar.copy(
                            outT[:, h0:h0 + 4, gs:gs + 128], ops_[:])

                def emit_oproj(tt):
                    for dc in range(4):
                        wo = cur["wo"][dc]
                        ps = psP.tile([128, TBLK], F32, tag="proj")
                        for et in range(ET):
                            nc.tensor.matmul(
                                ps[:], outT[:, et, tt * 128:(tt + 1) * 128],
                                wo[:, et, :],
                                start=(et == 0), stop=(et == ET - 1),
                            )
                        osb = osbp.tile([128, TBLK], BF16, tag="osb")
                        nc.scalar.copy(osb[:], ps[:])
                        nc.gpsimd.dma_start(
                            outd[ts + tt * 128: ts + (tt + 1) * 128,
                                 dc * 512:(dc + 1) * 512],
                            osb[:],
                        )

                emit_av(0)
                emit_av(1)
                emit_oproj(0)
                emit_av(2)
                emit_oproj(1)
                emit_av(3)
                emit_oproj(2)
                emit_oproj(3)

                cur = nxt
    return _patch_to_json(nc)


def _host_prep(x, rope_freqs, wq, wk, wv, wo):
    bf16 = ml_dtypes.bfloat16
    x_flat = np.ascontiguousarray(x, dtype=np.float32).reshape(B * S, D)
    wqTb = np.ascontiguousarray(wq.T).astype(bf16)
    wkTb = np.ascontiguousarray(wk.T).astype(bf16)
    wvTb = np.ascontiguousarray(wv.T).astype(bf16)
    woTb = np.ascontiguousarray(wo.T).astype(bf16)

    f = np.asarray(rope_freqs[:W], dtype=np.float32)  # [16, 64]
    cosf, sinf = np.cos(f), np.sin(f)                 # [16, 64]
    tmod = np.arange(TBLK) % W
    cs = np.empty((128, TBLK), np.float32)
    sn = np.empty((128, TBLK), np.float32)
    p = np.arange(128)
    cs[:, :] = cosf[tmod[None, :], (p % 64)[:, None]]
    sn[:, :] = sinf[tmod[None, :], (p % 64)[:, None]]
    sn[0:64, :] *= -1.0

    maskm = np.full((128, 128), MASK_NEG, np.float32)
    for wdw in range(128 // W):
        maskm[wdw * W:(wdw + 1) * W, wdw * W:(wdw + 1) * W] = 0.0
    maskm = np.repeat(maskm[:, None, :], 4, axis=1).copy()
    iden = np.eye(128, dtype=bf16)

    shared = dict(wqTb=wqTb, wkTb=wkTb, wvTb=wvTb, woTb=woTb,
                  csd=cs.astype(bf16), snd=sn.astype(bf16),
                  maskd=maskm.astype(bf16), idend=iden)
    in_maps = []
    for c in range(NCORES):
        shard = x_flat[c * TOK_PER_CORE:(c + 1) * TOK_PER_CORE]
        xT = np.ascontiguousarray(shard.T)
        in_maps.append(dict(shared, xTb=xT.astype(bf16)))
    return in_maps


@lru_cache(maxsize=1)
def _get_nc():
    return build_kernel()


def kernel(x, rope_freqs, wq, wk, wv, wo):
    in_maps = _host_prep(x, rope_freqs, wq, wk, wv, wo)
    nc = _get_nc()
    res = run_bass_kernel_spmd(
        nc, in_maps, core_ids=list(range(NCORES)),
        trace=bool(int(os.environ.get("LWA_TRACE", "0"))),
    )
    if getattr(kernel, "_last_results", None) is not None or True:
        kernel._last_results = res
    out = np.concatenate(
        [np.asarray(r["out"], dtype=np.float32) for r in res.results], axis=0)
    return out.reshape(B, S, D)
